# revision 30
# baseline (speedup 1.0000x reference)
"""Trainium2 Bass kernel for nn_HLSTransformer (2-block self-attention encoder).

Contract: kernel(**inputs) takes FULL inputs, returns FULL output [32, 1] f32.
Data-parallel over batch: 4 samples per core x 8 cores.

On-device layout: activations in "T layout" [H=64 partitions, N=1024 free],
two samples packed per 128-partition tile (rows 0-63 = even sample, 64-127 =
odd sample of the pair).

Attention design (per pair of samples, per block):
  - For each n-chunk c (8 chunks of 128 tokens) and sample s, one PSUM gram
    tile g = S_s[n in chunk, m in 0:1024] via two row-tiled matmuls (the two
    samples' matmuls run concurrently on disjoint PE row strips).
  - One big ACT exp per tile writes e (bf16 SBUF) and, via accum_out, the
    per-row partial sums = softmax denominators Z (scores are symmetric, so
    row sums equal the column sums needed later).  No max-subtract needed:
    softmax((s-max+mask)*SCALE) == softmax((s+mask)*SCALE) and |s*SCALE| is
    small enough that fp32 exp cannot overflow.
  - U = E @ h accumulated over chunks into two PSUM banks (m-halves); the two
    samples' U matmuls are column-tiled onto disjoint PE col strips and run
    concurrently.  U issue is delayed a few chunks so the previous pair's
    normalize can release the U banks without stalling the PE queue.
  - 1/Z: DVE reciprocal -> PE transpose -> SBUF-to-SBUF DMA gather into row
    form [2, 1024] -> one matmul pair against a {0,1} selector broadcasts it
    across partitions -> DVE multiply normalizes U.
  - LayerNorm over the whole [N, H] slab per sample -> scalar mean/var;
    rsqrt(var+eps) = exp(-0.5*ln(var+eps)) keeps ACT on one table set.

Emission interleaves the two pairs: while pair B's exp stream keeps the ACT
engine saturated, pair A's normalize/LN/FFN chain and next-block transposes
run on DVE/PE in the shadow.  ACT (the exp stream) is the roofline engine.
"""

import sys

import numpy as np
import ml_dtypes

if "/opt/trn_rl_repo" not in sys.path:
    sys.path.insert(0, "/opt/trn_rl_repo")

import concourse.bass as bass
import concourse.bacc as bacc
import concourse.tile as tile
from concourse import mybir
from concourse.bass_utils import run_bass_kernel_spmd

F32 = mybir.dt.float32
DT16 = mybir.dt.float16
FP8 = mybir.dt.float8e4
AF = mybir.ActivationFunctionType
ALU = mybir.AluOpType

USE_FP8_U = False          # fp8 DoubleRow for the U (= E @ h) matmuls

# Force Exp and Ln to resolve to the one table set containing both, so the
# ACT engine never thrashes ACT_TABLE_LOADs between them.
_orig_gat = bacc.get_activation_tables
def _gat_patched(arch):
    out = {}
    for name, fns in _orig_gat(arch).items():
        fns = set(fns)
        if name != "natural_log_exp_and_others":
            fns.discard(mybir.ActivationFunctionType.Exp)
            fns.discard(mybir.ActivationFunctionType.Ln)
        out[name] = fns
    return out
bacc.get_activation_tables = _gat_patched

B, N, F_IN, H = 32, 1024, 256, 64
NCORES = 8
S = B // NCORES            # samples per core
NPAIR = S // 2             # sample pairs per core
EPS = 1e-5
SCALE = float(1.0 / np.sqrt(np.float32(N)))
NH = 512                   # free-dim half (PSUM bank)
UDELAY = 3                 # U-matmul issue delay, in (chunk-pair, sample) units


def build_nc(use_mask: bool, use_gb: bool) -> bass.Bass:
    nc = bacc.Bacc("TRN2", target_bir_lowering=False, debug=False, num_devices=NCORES)

    EDT = FP8 if USE_FP8_U else DT16
    xT = nc.declare_dram_parameter("xT", [S, 2, 128, N], DT16, isOutput=False)
    we = nc.declare_dram_parameter("We", [2, 128, H], DT16, isOutput=False)
    w0 = nc.declare_dram_parameter("W0s", [128, H], DT16, isOutput=False)
    w1 = nc.declare_dram_parameter("W1s", [128, H], DT16, isOutput=False)
    wout = nc.declare_dram_parameter("Wouts", [128, 1], F32, isOutput=False)
    be2 = nc.declare_dram_parameter("be2", [128, 1], F32, isOutput=False)
    b02 = nc.declare_dram_parameter("b02", [128, 1], F32, isOutput=False)
    b12 = nc.declare_dram_parameter("b12", [128, 1], F32, isOutput=False)
    boutp = nc.declare_dram_parameter("bout", [1, 1], F32, isOutput=False)
    ident = nc.declare_dram_parameter("ident", [128, 128], DT16, isOutput=False)
    selbcb_d = nc.declare_dram_parameter("selbcb", [2, 128], DT16, isOutput=False)
    allsel_d = nc.declare_dram_parameter("allsel", [128, 128], F32, isOutput=False)
    if use_gb:
        gT2_d = nc.declare_dram_parameter("gT2", [128, N], F32, isOutput=False)
        bT2_d = nc.declare_dram_parameter("bT2", [128, N], F32, isOutput=False)
    if use_mask:
        maskN_d = nc.declare_dram_parameter("maskN", [S, N, N], F32, isOutput=False)
    out_d = nc.declare_dram_parameter("out", [S, 1], F32, isOutput=True)

    with tile.TileContext(nc) as tc:
        with (
            tc.tile_pool(name="consts", bufs=1) as cp,
            tc.tile_pool(name="xt", bufs=8) as xp,
            tc.tile_pool(name="big", bufs=2) as bigp,
            tc.tile_pool(name="hn", bufs=2) as hnp,
            tc.tile_pool(name="e", bufs=8) as ep,
            tc.tile_pool(name="small", bufs=2) as smp,
            tc.tile_pool(name="ring", bufs=2, space="PSUM") as pg,
            tc.tile_pool(name="pu", bufs=2, space="PSUM") as pu,
            tc.tile_pool(name="pm", bufs=2, space="PSUM") as pm,
        ):
            # ---- constants (embed-critical ones first for startup) ----
            we_sb = cp.tile([128, 2, H], DT16, tag="we", name="we_sb")
            nc.sync.dma_start(we_sb[:, :, :], we.rearrange("k p m -> p k m"))
            be_sb = cp.tile([128, 1], F32, tag="be", name="be_sb")
            nc.sync.dma_start(be_sb[:, :], be2[:, :])
            id_sb = cp.tile([128, 128], DT16, tag="id", name="id_sb")
            nc.sync.dma_start(id_sb[:, :], ident[:, :])
            w0_sb = cp.tile([128, H], DT16, tag="w0", name="w0_sb")
            w1_sb = cp.tile([128, H], DT16, tag="w1", name="w1_sb")
            wo_sb = cp.tile([128, 1], F32, tag="wo", name="wo_sb")
            b0_sb = cp.tile([128, 1], F32, tag="b0", name="b0_sb")
            b1_sb = cp.tile([128, 1], F32, tag="b1", name="b1_sb")
            bo_sb = cp.tile([1, 1], F32, tag="bo", name="bo_sb")
            selbcb = cp.tile([2, 128], DT16, tag="sbb", name="selbcb_sb")
            allsel = cp.tile([128, 128], F32, tag="asel", name="allsel_sb")
            eps128 = cp.tile([128, 1], F32, tag="eps", name="eps128")
            nc.vector.memset(eps128[:, :], EPS)
            gb = None
            if use_gb:
                gam = cp.tile([128, N], F32, tag="gam", name="gam")
                bet = cp.tile([128, N], F32, tag="bet", name="bet")
                gb = (gam, bet)

            def load_rest_consts():
                nc.sync.dma_start(w0_sb[:, :], w0[:, :])
                nc.sync.dma_start(w1_sb[:, :], w1[:, :])
                nc.sync.dma_start(wo_sb[:, :], wout[:, :])
                nc.sync.dma_start(b0_sb[:, :], b02[:, :])
                nc.sync.dma_start(b1_sb[:, :], b12[:, :])
                nc.sync.dma_start(bo_sb[:, :], boutp[:, :])
                nc.sync.dma_start(selbcb[:, :], selbcb_d[:, :])
                nc.sync.dma_start(allsel[:, :], allsel_d[:, :])
                if use_gb:
                    nc.sync.dma_start(gam[:, :], gT2_d[:, :])
                    nc.sync.dma_start(bet[:, :], bT2_d[:, :])

            pair_state = [dict() for _ in range(NPAIR)]

            # ---- LayerNorm pieces ----
            # Per-sample scalar mean/var over the [N, H] slab.  Per-partition
            # (sum, sqsum) on DVE, then ONE matmul against a constant
            # block-diagonal (1/65536)-matrix performs the per-sample-group
            # partition reduce, the broadcast, and the averaging at once.
            def ln_stats(v, nm):
                zs = smp.tile([128, 2], F32, tag="zs", name=f"zs_{nm}")
                scra = bigp.tile([128, N], DT16, tag="scr", name=f"scra_{nm}")
                nc.vector.tensor_scalar(
                    scra[:, :], v[:, :], 1.0, 0.0, op0=ALU.mult, op1=ALU.add,
                    accum_out=zs[:, 0:1],
                )
                scr = bigp.tile([128, N], DT16, tag="scr", name=f"scr_{nm}")
                nc.vector.scalar_tensor_tensor(
                    scr[:, :], v[:, :], 1.0, v[:, :],
                    op0=ALU.mult, op1=ALU.mult, accum_out=zs[:, 1:2],
                )
                mrp = pm.tile([128, 2], F32, tag="m", name=f"mrp_{nm}")
                nc.tensor.matmul(mrp[:, :], allsel[:, :], zs[:, :],
                                 start=True, stop=True)
                mrs = smp.tile([128, 2], F32, tag="mrs", name=f"mrs_{nm}")
                nc.vector.tensor_copy(mrs[:, :], mrp[:, :])
                return mrs

            def ln_finish(v, mrs, out, nm):
                """nvar = mean^2 - ex2 (= -var); rho = exp(-0.5*ln(var+eps));
                out = (v - mean) * rho.  Two tiny ACT ops."""
                nvar = smp.tile([128, 1], F32, tag="nvar", name=f"nvar_{nm}")
                nc.vector.scalar_tensor_tensor(
                    nvar[:, :], mrs[:, 0:1], mrs[:, 0:1], mrs[:, 1:2],
                    op0=ALU.mult, op1=ALU.subtract,
                )
                lnv = smp.tile([128, 1], F32, tag="lnv", name=f"lnv_{nm}")
                nc.scalar.activation(lnv[:, :], nvar[:, :], AF.Ln, scale=-1.0,
                                     bias=eps128[:, :])
                rho = smp.tile([128, 1], F32, tag="rho", name=f"rho_{nm}")
                nc.scalar.activation(rho[:, :], lnv[:, :], AF.Exp, scale=-0.5)
                nc.vector.tensor_scalar(
                    out[:, :], v[:, :], mrs[:, 0:1], rho[:, 0:1],
                    op0=ALU.subtract, op1=ALU.mult,
                )
                if gb is not None:
                    nc.vector.tensor_tensor(out[:, :], out[:, :], gb[0][:, :], op=ALU.mult)
                    nc.vector.tensor_tensor(out[:, :], out[:, :], gb[1][:, :], op=ALU.add)

            # ---- transposes: hT [128, 1024] -> hnc [128, 8, 128] ----
            def emit_transposes(hT_src, nm):
                tp = pm.tile([128, 8, 128], DT16, tag="m", name=f"tp_{nm}")
                for c in range(8):
                    nc.tensor.transpose(
                        tp[:, c, :], hT_src[:, 128 * c:128 * c + 128], id_sb[:, :],
                    )
                hnc = hnp.tile([128, 8, 128], EDT, tag="hn", name=f"hnc_{nm}")
                nc.vector.tensor_copy(hnc[:, :, :], tp[:, :, :])
                return hnc

            # ---- embed: x_embT = relu(We.T @ xT + be) ----
            # x loads ride the (otherwise idle at startup) DVE/GpSimd DMA
            # trigger queues so they don't serialize behind const loads.
            def emit_embed(p):
                xts = []
                for si in range(2):
                    for k in range(2):
                        t = xp.tile([128, N], DT16, tag="xt", name=f"x_{p}_{si}_{k}")
                        trig = nc.gpsimd if si == 0 else nc.sync
                        trig.dma_start(t[:, :], xT[2 * p + si, k, :, :])
                        xts.append((si, k, t))
                xe = bigp.tile([128, N], DT16, tag="xemb", name=f"xe_{p}")
                for half in range(2):
                    cols = slice(NH * half, NH * half + NH)
                    emb = pm.tile([128, NH], F32, tag="m", name=f"emb_{p}_{half}")
                    for (si, k, t) in xts:
                        nc.tensor.matmul(
                            emb[64 * si:64 * si + 64, :], we_sb[:, k, :], t[:, cols],
                            start=(k == 0), stop=(k == 1),
                            tile_position=(0, 64 * si), skip_group_check=True,
                        )
                    nc.vector.tensor_scalar(
                        xe[:, cols], emb[:, :], be_sb[:, :], 0.0,
                        op0=ALU.add, op1=ALU.max,
                    )
                pair_state[p]["xemb"] = xe
                pair_state[p]["hT"] = xe

            # ---- attention phase for (pair, block) ----
            def emit_attention(p, b, hooks):
                st = pair_state[p]
                hT = st["hT"]
                hnc = st["hnc"]
                zacc = smp.tile([128, 16], F32, tag="zacc", name=f"zacc_{p}_{b}")
                u_lo = pu.tile([128, NH], F32, tag="u", name=f"ulo_{p}_{b}")
                u_hi = pu.tile([128, NH], F32, tag="u", name=f"uhi_{p}_{b}")
                pend = []

                def emit_u(item):
                    # one DoubleRow matmul contracts a pair of n-chunks
                    cp_, s, e2 = item
                    r0 = 64 * s
                    lhsT = hnc[:, 2 * cp_:2 * cp_ + 2, r0:r0 + 64]
                    for u, half in ((u_lo, slice(0, NH)), (u_hi, slice(NH, N))):
                        if USE_FP8_U:
                            nc.tensor.matmul(
                                u[r0:r0 + 64, :], lhsT, e2[:, :, half],
                                start=(cp_ == 0), stop=(cp_ == 3),
                                perf_mode=mybir.MatmulPerfMode.DoubleRow,
                                skip_group_check=True,
                            )
                        else:
                            for o in range(2):
                                nc.tensor.matmul(
                                    u[r0:r0 + 64, :], hnc[:, 2 * cp_ + o, r0:r0 + 64],
                                    e2[:, o, half],
                                    start=(cp_ == 0 and o == 0),
                                    stop=(cp_ == 3 and o == 1),
                                    skip_group_check=True,
                                )

                e2cur = [None, None]
                for c in range(8):
                    for s in range(2):
                        g = pg.tile([128, N], F32, tag="g", name=f"g_{p}_{b}_{c}_{s}")
                        r0 = 64 * s
                        lhs = hT[r0:r0 + 64, 128 * c:128 * c + 128]
                        nc.tensor.matmul(g[:, 0:NH], lhs, hT[r0:r0 + 64, 0:NH],
                                         start=True, stop=True)
                        nc.tensor.matmul(g[:, NH:N], lhs, hT[r0:r0 + 64, NH:N],
                                         start=True, stop=True)
                        if use_mask:
                            mt = ep.tile([128, N], F32, tag="mt", bufs=2,
                                         name=f"mt_{p}_{b}_{c}_{s}")
                            nc.sync.dma_start(
                                mt[:, :],
                                maskN_d[2 * p + s, 128 * c:128 * c + 128, :],
                            )
                            nc.vector.tensor_tensor(g[:, :], g[:, :], mt[:, :],
                                                    op=ALU.add)
                        if c % 2 == 0:
                            e2cur[s] = ep.tile([128, 2, N], EDT, tag="e", bufs=8,
                                               name=f"e_{p}_{b}_{c}_{s}")
                        e2 = e2cur[s]
                        col = 8 * s + c
                        # Z row-partials: split between the ACT accumulator
                        # (s=0) and a DVE 4x-mode pass (s=1) to balance load.
                        if s == 0:
                            nc.scalar.activation(e2[:, c % 2, :], g[:, :], AF.Exp,
                                                 scale=SCALE,
                                                 accum_out=zacc[:, col:col + 1])
                        else:
                            nc.scalar.activation(e2[:, c % 2, :], g[:, :], AF.Exp,
                                                 scale=SCALE)
                            zscr = ep.tile([128, N], EDT, tag="zscr", bufs=2,
                                           name=f"zscr_{p}_{b}_{c}_{s}")
                            nc.vector.tensor_scalar(
                                zscr[:, :], e2[:, c % 2, :], 1.0, 0.0,
                                op0=ALU.mult, op1=ALU.add,
                                accum_out=zacc[:, col:col + 1],
                            )
                        if c % 2 == 1:
                            pend.append((c // 2, s, e2))
                            while len(pend) > UDELAY:
                                emit_u(pend.pop(0))
                    if c in hooks:
                        hooks[c]()
                while pend:
                    emit_u(pend.pop(0))
                return zacc, u_lo, u_hi

            # ---- post-attention chain, split for interleaved emission ----
            def make_post(p, b, zacc, u_lo, u_hi):
                st = {}
                wf = w0_sb if b == 0 else w1_sb
                bf = b0_sb if b == 0 else b1_sb

                def a1():
                    # 1/Z and its journey to row form (no ACT ops)
                    with nc.allow_low_precision("1/Z in bf16 is plenty"):
                        rzc = smp.tile([128, 16], DT16, tag="rzc",
                                       name=f"rzc_{p}_{b}")
                        nc.vector.reciprocal(rzc[:, :], zacc[:, :])
                    zrt = pm.tile([16, 128], DT16, tag="m", name=f"zrt_{p}_{b}")
                    nc.tensor.transpose(zrt[:, :], rzc[:, :], id_sb[:, :])
                    rzt = smp.tile([16, 128], DT16, tag="rzt", name=f"rzt_{p}_{b}")
                    nc.vector.tensor_copy(rzt[:, :], zrt[:, :])
                    rzrow = smp.tile([2, 8, 128], DT16, tag="rzrow",
                                     name=f"rzrow_{p}_{b}")
                    nc.sync.dma_start(rzrow[0:1, :, :], rzt[0:8, :])
                    nc.sync.dma_start(rzrow[1:2, :, :], rzt[8:16, :])
                    st["rzrow"] = rzrow

                def a2():
                    # broadcast 1/Z across partitions, normalize U, residual,
                    # then qkv LN stats (no ACT ops)
                    rzrow2 = st["rzrow"].rearrange("s c j -> s (c j)")
                    rzb_lo = pm.tile([128, NH], F32, tag="m", name=f"rzblo_{p}_{b}")
                    rzb_hi = pm.tile([128, NH], F32, tag="m", name=f"rzbhi_{p}_{b}")
                    nc.tensor.matmul(rzb_lo[:, :], selbcb[:, :], rzrow2[:, 0:NH],
                                     start=True, stop=True)
                    nc.tensor.matmul(rzb_hi[:, :], selbcb[:, :], rzrow2[:, NH:N],
                                     start=True, stop=True)
                    # DVE reads at most one PSUM operand per instruction, so
                    # stage 1/Z in SBUF before the U multiply.
                    rzb = smp.tile([128, N], F32, tag="rzb", name=f"rzb_{p}_{b}")
                    nc.vector.tensor_copy(rzb[:, 0:NH], rzb_lo[:, :])
                    nc.vector.tensor_copy(rzb[:, NH:N], rzb_hi[:, :])
                    v = bigp.tile([128, N], DT16, tag="v", name=f"v_{p}_{b}")
                    nc.vector.tensor_tensor(v[:, 0:NH], u_lo[:, :], rzb[:, 0:NH],
                                            op=ALU.mult)
                    nc.vector.tensor_tensor(v[:, NH:N], u_hi[:, :], rzb[:, NH:N],
                                            op=ALU.mult)
                    nc.vector.tensor_tensor(v[:, :], v[:, :],
                                            pair_state[p]["xemb"][:, :], op=ALU.add)
                    st["v"] = v
                    st["mrs1"] = ln_stats(v, f"q{p}{b}")

                def bfn():
                    # qkv LN finish (2 tiny ACT ops), FFN, fc LN stats
                    qkv = bigp.tile([128, N], DT16, tag="qkv", name=f"qkv_{p}_{b}")
                    ln_finish(st["v"], st["mrs1"], qkv, f"q{p}{b}")
                    fps_lo = pm.tile([128, NH], F32, tag="m", name=f"fpslo_{p}_{b}")
                    fps_hi = pm.tile([128, NH], F32, tag="m", name=f"fpshi_{p}_{b}")
                    for fps, cols in ((fps_lo, slice(0, NH)), (fps_hi, slice(NH, N))):
                        nc.tensor.matmul(fps[0:64, :], wf[0:64, :], qkv[0:64, cols],
                                         start=True, stop=True)
                        nc.tensor.matmul(fps[64:128, :], wf[64:128, :],
                                         qkv[64:128, cols], start=True, stop=True)
                    f = bigp.tile([128, N], DT16, tag="f", name=f"f_{p}_{b}")
                    nc.vector.tensor_scalar(f[:, 0:NH], fps_lo[:, :], bf[:, :], 0.0,
                                            op0=ALU.add, op1=ALU.max)
                    nc.vector.tensor_scalar(f[:, NH:N], fps_hi[:, :], bf[:, :], 0.0,
                                            op0=ALU.add, op1=ALU.max)
                    nc.vector.tensor_tensor(f[:, :], f[:, :], qkv[:, :], op=ALU.add)
                    st["f"] = f
                    st["mrs2"] = ln_stats(f, f"f{p}{b}")

                def cfn():
                    # fc LN finish (2 tiny ACT ops), then either next-block
                    # transposes or the output head
                    fc = bigp.tile([128, N], DT16, tag="fc", name=f"fc_{p}_{b}")
                    ln_finish(st["f"], st["mrs2"], fc, f"f{p}{b}")
                    pair_state[p]["hT"] = fc
                    if b == 0:
                        pair_state[p]["hnc"] = emit_transposes(fc, f"{p}_{b + 1}")
                    else:
                        emit_head(p, fc)

                return a1, a2, bfn, cfn

            # ---- pool + head ----
            def emit_head(p, fc):
                pooled = smp.tile([128, 1], F32, tag="pool", name=f"pool_{p}")
                nc.vector.reduce_sum(pooled[:, :], fc[:, :],
                                     axis=mybir.AxisListType.X)
                sc = pm.tile([1, 2], F32, tag="m", name=f"sc_{p}")
                nc.tensor.matmul(sc[0:1, 0:1], wo_sb[0:64, :], pooled[0:64, :],
                                 start=True, stop=True, skip_group_check=True)
                nc.tensor.matmul(sc[0:1, 1:2], wo_sb[64:128, :], pooled[64:128, :],
                                 start=True, stop=True, skip_group_check=True)
                zb = smp.tile([1, 2], F32, tag="zb", name=f"zb_{p}")
                nc.vector.tensor_scalar(zb[:, :], sc[:, :], bo_sb[0:1, :], None,
                                        op0=ALU.add)
                res = smp.tile([1, 2], F32, tag="res", name=f"res_{p}")
                nc.vector.tensor_scalar(res[:, :], zb[:, :], 0.01, None,
                                        op0=ALU.mult)
                nc.vector.tensor_tensor(res[:, :], res[:, :], zb[:, :], op=ALU.max)
                for si in range(2):
                    nc.sync.dma_start(
                        out_d[2 * p + si:2 * p + si + 1, :], res[0:1, si:si + 1],
                    )

            # ---- main schedule ----
            emit_embed(0)
            load_rest_consts()
            pair_state[0]["hnc"] = emit_transposes(pair_state[0]["hT"], "0_0")
            emit_embed(1)

            phases = [(0, 0), (1, 0), (0, 1), (1, 1)]
            pending = None
            for i, (p, b) in enumerate(phases):
                hooks = {}
                if pending is not None:
                    a1, a2, bfn, cfn = pending
                    hooks[0] = a1
                    hooks[1] = a2
                    hooks[3] = bfn
                    hooks[5] = cfn
                if i == 0:
                    hooks[2] = lambda: pair_state[1].__setitem__(
                        "hnc", emit_transposes(pair_state[1]["hT"], "1_0"))
                zacc, u_lo, u_hi = emit_attention(p, b, hooks)
                pending = make_post(p, b, zacc, u_lo, u_hi)

            for fn in pending:
                fn()

    nc.compile()
    return nc


_NC_CACHE: dict = {}


def prepare_common(We, be, gamma, beta, W0, b0, W1, b1, Wout, bout, use_gb):
    ident = np.eye(128, dtype=np.float32)
    selbc = np.zeros((2, 128), dtype=np.float32)
    selbc[0, 0:64] = 1.0
    selbc[1, 64:128] = 1.0

    def stack2(v):
        v = np.asarray(v, dtype=np.float32).reshape(-1)
        return np.concatenate([v, v]).reshape(128, 1)

    common = {
        "We": np.ascontiguousarray(np.asarray(We, dtype=np.float32)).reshape(
            2, 128, H).astype(np.float16),
        "W0s": np.concatenate([W0, W0]).astype(np.float16),
        "W1s": np.concatenate([W1, W1]).astype(np.float16),
        "Wouts": np.concatenate([Wout, Wout]).astype(np.float32),
        "be2": stack2(be), "b02": stack2(b0), "b12": stack2(b1),
        "bout": np.asarray(bout, dtype=np.float32).reshape(1, 1),
        "ident": ident.astype(np.float16),
        "selbcb": selbc.astype(np.float16),
        "allsel": (np.kron(np.eye(2, dtype=np.float32),
                           np.ones((64, 64), dtype=np.float32)) / 65536.0),
    }
    if use_gb:
        gT = np.ascontiguousarray(np.asarray(gamma, dtype=np.float32).T)
        bT = np.ascontiguousarray(np.asarray(beta, dtype=np.float32).T)
        common["gT2"] = np.concatenate([gT, gT]).astype(np.float32)
        common["bT2"] = np.concatenate([bT, bT]).astype(np.float32)
    return common


def prepare_core_map(common, x, mask, k, use_mask):
    xs = x[S * k:S * k + S]                       # [S, N, F_IN]
    xTs = np.ascontiguousarray(xs.transpose(0, 2, 1)).reshape(S, 2, 128, N)
    m = dict(common)
    m["xT"] = xTs.astype(np.float16)
    if use_mask:
        m["maskN"] = np.ascontiguousarray(mask[S * k:S * k + S])
    return m


def kernel(x, mask, We, be, gamma, beta, W0, b0, W1, b1, Wout, bout):
    x = np.ascontiguousarray(np.asarray(x, dtype=np.float32))
    mask = np.asarray(mask, dtype=np.float32)
    use_mask = bool(np.any(mask))
    use_gb = bool(np.any(np.asarray(gamma) != 1.0) or np.any(np.asarray(beta)))

    key = (use_mask, use_gb)
    if key not in _NC_CACHE:
        _NC_CACHE[key] = build_nc(use_mask, use_gb)
    nc = _NC_CACHE[key]

    common = prepare_common(We, be, gamma, beta, W0, b0, W1, b1, Wout, bout,
                            use_gb)
    in_maps = [prepare_core_map(common, x, mask, k, use_mask)
               for k in range(NCORES)]

    res = run_bass_kernel_spmd(nc, in_maps, list(range(NCORES)))
    global LAST_RESULT
    LAST_RESULT = res
    out = np.concatenate([res.results[k]["out"] for k in range(NCORES)], axis=0)
    return out.astype(np.float32)


LAST_RESULT = None


# revision 32
# speedup vs baseline: 1.0562x; 1.0562x over previous
"""Trainium2 Bass kernel for nn_HLSTransformer (2-block self-attention encoder).

Contract: kernel(**inputs) takes FULL inputs, returns FULL output [32, 1] f32.
Data-parallel over batch: 4 samples per core x 8 cores.

On-device layout: activations in "T layout" [H=64 partitions, N=1024 free],
two samples packed per 128-partition tile (rows 0-63 = even sample, 64-127 =
odd sample of the pair).

Attention design (per pair of samples, per block):
  - For each n-chunk c (8 chunks of 128 tokens) and sample s, one PSUM gram
    tile g = S_s[n in chunk, m in 0:1024] via two row-tiled matmuls (the two
    samples' matmuls run concurrently on disjoint PE row strips).
  - One big ACT exp per tile writes e (bf16 SBUF) and, via accum_out, the
    per-row partial sums = softmax denominators Z (scores are symmetric, so
    row sums equal the column sums needed later).  No max-subtract needed:
    softmax((s-max+mask)*SCALE) == softmax((s+mask)*SCALE) and |s*SCALE| is
    small enough that fp32 exp cannot overflow.
  - U = E @ h accumulated over chunks into two PSUM banks (m-halves); the two
    samples' U matmuls are column-tiled onto disjoint PE col strips and run
    concurrently.  U issue is delayed a few chunks so the previous pair's
    normalize can release the U banks without stalling the PE queue.
  - 1/Z: DVE reciprocal -> PE transpose -> SBUF-to-SBUF DMA gather into row
    form [2, 1024] -> one matmul pair against a {0,1} selector broadcasts it
    across partitions -> DVE multiply normalizes U.
  - LayerNorm over the whole [N, H] slab per sample -> scalar mean/var;
    rsqrt(var+eps) = exp(-0.5*ln(var+eps)) keeps ACT on one table set.

Emission interleaves the two pairs: while pair B's exp stream keeps the ACT
engine saturated, pair A's normalize/LN/FFN chain and next-block transposes
run on DVE/PE in the shadow.  ACT (the exp stream) is the roofline engine.
"""

import sys

import numpy as np
import ml_dtypes

if "/opt/trn_rl_repo" not in sys.path:
    sys.path.insert(0, "/opt/trn_rl_repo")

import concourse.bass as bass
import concourse.bacc as bacc
import concourse.tile as tile
from concourse import mybir
from concourse.bass_utils import run_bass_kernel_spmd

F32 = mybir.dt.float32
DT16 = mybir.dt.float16
FP8 = mybir.dt.float8e4
AF = mybir.ActivationFunctionType
ALU = mybir.AluOpType

USE_FP8_U = False          # fp8 DoubleRow for the U (= E @ h) matmuls

# Force Exp and Ln to resolve to the one table set containing both, so the
# ACT engine never thrashes ACT_TABLE_LOADs between them.
_orig_gat = bacc.get_activation_tables
def _gat_patched(arch):
    out = {}
    for name, fns in _orig_gat(arch).items():
        fns = set(fns)
        if name != "natural_log_exp_and_others":
            fns.discard(mybir.ActivationFunctionType.Exp)
            fns.discard(mybir.ActivationFunctionType.Ln)
        out[name] = fns
    return out
bacc.get_activation_tables = _gat_patched

B, N, F_IN, H = 32, 1024, 256, 64
NCORES = 8
S = B // NCORES            # samples per core
NPAIR = S // 2             # sample pairs per core
EPS = 1e-5
SCALE = float(1.0 / np.sqrt(np.float32(N)))
NH = 512                   # free-dim half (PSUM bank)
UDELAY = 1                 # U-matmul issue delay, in chunk-pair units


def build_nc(use_mask: bool, use_gb: bool) -> bass.Bass:
    nc = bacc.Bacc("TRN2", target_bir_lowering=False, debug=False, num_devices=NCORES)

    EDT = FP8 if USE_FP8_U else DT16
    xT = nc.declare_dram_parameter("xT", [S, 2, 128, N], DT16, isOutput=False)
    we = nc.declare_dram_parameter("We", [2, 128, H], DT16, isOutput=False)
    w0 = nc.declare_dram_parameter("W0s", [128, H], DT16, isOutput=False)
    w1 = nc.declare_dram_parameter("W1s", [128, H], DT16, isOutput=False)
    wout = nc.declare_dram_parameter("Wouts", [128, 1], F32, isOutput=False)
    be2 = nc.declare_dram_parameter("be2", [128, 1], F32, isOutput=False)
    b02 = nc.declare_dram_parameter("b02", [128, 1], F32, isOutput=False)
    b12 = nc.declare_dram_parameter("b12", [128, 1], F32, isOutput=False)
    boutp = nc.declare_dram_parameter("bout", [1, 1], F32, isOutput=False)
    ident = nc.declare_dram_parameter("ident", [128, 128], DT16, isOutput=False)
    selbcb_d = nc.declare_dram_parameter("selbcb", [2, 128], DT16, isOutput=False)
    allsel_d = nc.declare_dram_parameter("allsel", [128, 128], F32, isOutput=False)
    if use_gb:
        gT2_d = nc.declare_dram_parameter("gT2", [128, N], F32, isOutput=False)
        bT2_d = nc.declare_dram_parameter("bT2", [128, N], F32, isOutput=False)
    if use_mask:
        maskN_d = nc.declare_dram_parameter("maskN", [S, N, N], F32, isOutput=False)
    out_d = nc.declare_dram_parameter("out", [S, 1], F32, isOutput=True)

    with tile.TileContext(nc) as tc:
        with (
            tc.tile_pool(name="consts", bufs=1) as cp,
            tc.tile_pool(name="xt", bufs=8) as xp,
            tc.tile_pool(name="big", bufs=2) as bigp,
            tc.tile_pool(name="hn", bufs=2) as hnp,
            tc.tile_pool(name="e", bufs=8) as ep,
            tc.tile_pool(name="small", bufs=2) as smp,
            tc.tile_pool(name="ring", bufs=2, space="PSUM") as pg,
            tc.tile_pool(name="pu", bufs=2, space="PSUM") as pu,
            tc.tile_pool(name="pm", bufs=2, space="PSUM") as pm,
        ):
            # ---- constants (embed-critical ones first for startup) ----
            we_sb = cp.tile([128, 2, H], DT16, tag="we", name="we_sb")
            nc.sync.dma_start(we_sb[:, :, :], we.rearrange("k p m -> p k m"))
            be_sb = cp.tile([128, 1], F32, tag="be", name="be_sb")
            nc.sync.dma_start(be_sb[:, :], be2[:, :])
            id_sb = cp.tile([128, 128], DT16, tag="id", name="id_sb")
            nc.sync.dma_start(id_sb[:, :], ident[:, :])
            w0_sb = cp.tile([128, H], DT16, tag="w0", name="w0_sb")
            w1_sb = cp.tile([128, H], DT16, tag="w1", name="w1_sb")
            wo_sb = cp.tile([128, 1], F32, tag="wo", name="wo_sb")
            b0_sb = cp.tile([128, 1], F32, tag="b0", name="b0_sb")
            b1_sb = cp.tile([128, 1], F32, tag="b1", name="b1_sb")
            bo_sb = cp.tile([1, 1], F32, tag="bo", name="bo_sb")
            selbcb = cp.tile([2, 128], DT16, tag="sbb", name="selbcb_sb")
            allsel = cp.tile([128, 128], F32, tag="asel", name="allsel_sb")
            eps128 = cp.tile([128, 1], F32, tag="eps", name="eps128")
            nc.vector.memset(eps128[:, :], EPS)
            gb = None
            if use_gb:
                gam = cp.tile([128, N], F32, tag="gam", name="gam")
                bet = cp.tile([128, N], F32, tag="bet", name="bet")
                gb = (gam, bet)

            def load_rest_consts():
                nc.sync.dma_start(w0_sb[:, :], w0[:, :])
                nc.sync.dma_start(w1_sb[:, :], w1[:, :])
                nc.sync.dma_start(wo_sb[:, :], wout[:, :])
                nc.sync.dma_start(b0_sb[:, :], b02[:, :])
                nc.sync.dma_start(b1_sb[:, :], b12[:, :])
                nc.sync.dma_start(bo_sb[:, :], boutp[:, :])
                nc.sync.dma_start(selbcb[:, :], selbcb_d[:, :])
                nc.sync.dma_start(allsel[:, :], allsel_d[:, :])
                if use_gb:
                    nc.sync.dma_start(gam[:, :], gT2_d[:, :])
                    nc.sync.dma_start(bet[:, :], bT2_d[:, :])

            pair_state = [dict() for _ in range(NPAIR)]

            # ---- LayerNorm pieces ----
            # Per-sample scalar mean/var over the [N, H] slab.  Per-partition
            # (sum, sqsum) on DVE, then ONE matmul against a constant
            # block-diagonal (1/65536)-matrix performs the per-sample-group
            # partition reduce, the broadcast, and the averaging at once.
            def ln_stats(v, nm):
                st6 = smp.tile([128, 12], F32, tag="st6", name=f"st6_{nm}")
                nc.vector.bn_stats(st6[:, 0:6], v[:, 0:NH])
                nc.vector.bn_stats(st6[:, 6:12], v[:, NH:N])
                ag = smp.tile([128, 2], F32, tag="ag", name=f"ag_{nm}")
                nc.vector.bn_aggr(ag[:, :], st6[:, :])
                zs = smp.tile([128, 2], F32, tag="zs", name=f"zs_{nm}")
                nc.vector.tensor_copy(zs[:, 0:1], ag[:, 0:1])
                nc.vector.scalar_tensor_tensor(
                    zs[:, 1:2], ag[:, 0:1], ag[:, 0:1], ag[:, 1:2],
                    op0=ALU.mult, op1=ALU.add,
                )
                mrp = pm.tile([128, 2], F32, tag="m", name=f"mrp_{nm}")
                nc.tensor.matmul(mrp[:, :], allsel[:, :], zs[:, :],
                                 start=True, stop=True)
                mrs = smp.tile([128, 2], F32, tag="mrs", name=f"mrs_{nm}")
                nc.vector.tensor_copy(mrs[:, :], mrp[:, :])
                return mrs

            def ln_finish(v, mrs, out, nm):
                """nvar = mean^2 - ex2 (= -var); rho = exp(-0.5*ln(var+eps));
                out = (v - mean) * rho.  Two tiny ACT ops."""
                nvar = smp.tile([128, 1], F32, tag="nvar", name=f"nvar_{nm}")
                nc.vector.scalar_tensor_tensor(
                    nvar[:, :], mrs[:, 0:1], mrs[:, 0:1], mrs[:, 1:2],
                    op0=ALU.mult, op1=ALU.subtract,
                )
                lnv = smp.tile([128, 1], F32, tag="lnv", name=f"lnv_{nm}")
                nc.scalar.activation(lnv[:, :], nvar[:, :], AF.Ln, scale=-1.0,
                                     bias=eps128[:, :])
                rho = smp.tile([128, 1], F32, tag="rho", name=f"rho_{nm}")
                nc.scalar.activation(rho[:, :], lnv[:, :], AF.Exp, scale=-0.5)
                nc.vector.tensor_scalar(
                    out[:, :], v[:, :], mrs[:, 0:1], rho[:, 0:1],
                    op0=ALU.subtract, op1=ALU.mult,
                )
                if gb is not None:
                    nc.vector.tensor_tensor(out[:, :], out[:, :], gb[0][:, :], op=ALU.mult)
                    nc.vector.tensor_tensor(out[:, :], out[:, :], gb[1][:, :], op=ALU.add)

            # ---- transposes: hT [128, 1024] -> hnc [128, 8, 128] ----
            def emit_transposes(hT_src, nm):
                tp = pm.tile([128, 8, 128], DT16, tag="m", name=f"tp_{nm}")
                for c in range(8):
                    nc.tensor.transpose(
                        tp[:, c, :], hT_src[:, 128 * c:128 * c + 128], id_sb[:, :],
                    )
                hnc = hnp.tile([128, 8, 128], EDT, tag="hn", name=f"hnc_{nm}")
                nc.vector.tensor_copy(hnc[:, :, :], tp[:, :, :])
                return hnc

            # ---- embed: x_embT = relu(We.T @ xT + be) ----
            # x loads ride the (otherwise idle at startup) DVE/GpSimd DMA
            # trigger queues so they don't serialize behind const loads.
            def emit_embed(p):
                xts = []
                for si in range(2):
                    for k in range(2):
                        t = xp.tile([128, N], DT16, tag="xt", name=f"x_{p}_{si}_{k}")
                        trig = nc.gpsimd if si == 0 else nc.sync
                        trig.dma_start(t[:, :], xT[2 * p + si, k, :, :])
                        xts.append((si, k, t))
                xe = bigp.tile([128, N], DT16, tag="xemb", name=f"xe_{p}")
                for half in range(2):
                    cols = slice(NH * half, NH * half + NH)
                    emb = pm.tile([128, NH], F32, tag="m", name=f"emb_{p}_{half}")
                    for (si, k, t) in xts:
                        nc.tensor.matmul(
                            emb[64 * si:64 * si + 64, :], we_sb[:, k, :], t[:, cols],
                            start=(k == 0), stop=(k == 1),
                            tile_position=(0, 64 * si), skip_group_check=True,
                        )
                    for si in range(2):
                        r0 = 64 * si
                        nc.vector.tensor_scalar(
                            xe[r0:r0 + 64, cols], emb[r0:r0 + 64, :],
                            be_sb[r0:r0 + 64, :], 0.0,
                            op0=ALU.add, op1=ALU.max,
                        )
                pair_state[p]["xemb"] = xe
                pair_state[p]["hT"] = xe

            # ---- attention phase for (pair, block) ----
            def emit_attention(p, b, hooks):
                st = pair_state[p]
                hT = st["hT"]
                hnc = st["hnc"]
                zacc = smp.tile([128, 16], F32, tag="zacc", name=f"zacc_{p}_{b}")
                u_lo = pu.tile([128, NH], F32, tag="u", name=f"ulo_{p}_{b}")
                u_hi = pu.tile([128, NH], F32, tag="u", name=f"uhi_{p}_{b}")
                pend = []

                def emit_u(item):
                    # col-tiled sample pairs issued adjacently so they run
                    # concurrently on disjoint PE col strips
                    cp_, e2s = item
                    for u, half in ((u_lo, slice(0, NH)), (u_hi, slice(NH, N))):
                        for o in range(2):
                            for s in range(2):
                                r0 = 64 * s
                                nc.tensor.matmul(
                                    u[r0:r0 + 64, :],
                                    hnc[:, 2 * cp_ + o, r0:r0 + 64],
                                    e2s[s][:, o, half],
                                    start=(cp_ == 0 and o == 0),
                                    stop=(cp_ == 3 and o == 1),
                                    skip_group_check=True,
                                )

                e2cur = [None, None]
                for c in range(8):
                    gs = []
                    for s in range(2):
                        g = pg.tile([128, N], F32, tag="g", name=f"g_{p}_{b}_{c}_{s}")
                        gs.append(g)
                    # row-tiled sample pairs issued adjacently -> concurrent
                    for half in (slice(0, NH), slice(NH, N)):
                        for s in range(2):
                            r0 = 64 * s
                            nc.tensor.matmul(
                                gs[s][:, half],
                                hT[r0:r0 + 64, 128 * c:128 * c + 128],
                                hT[r0:r0 + 64, half],
                                start=True, stop=True,
                            )
                    for s in range(2):
                        g = gs[s]
                        if use_mask:
                            mt = ep.tile([128, N], F32, tag="mt", bufs=2,
                                         name=f"mt_{p}_{b}_{c}_{s}")
                            nc.sync.dma_start(
                                mt[:, :],
                                maskN_d[2 * p + s, 128 * c:128 * c + 128, :],
                            )
                            nc.vector.tensor_tensor(g[:, :], g[:, :], mt[:, :],
                                                    op=ALU.add)
                        if c % 2 == 0:
                            e2cur[s] = ep.tile([128, 2, N], EDT, tag="e", bufs=8,
                                               name=f"e_{p}_{b}_{c}_{s}")
                        col = 8 * s + c
                        nc.scalar.activation(
                            e2cur[s][:, c % 2, :], g[:, :], AF.Exp, scale=SCALE,
                            accum_out=zacc[:, col:col + 1],
                        )
                    if c % 2 == 1:
                        pend.append((c // 2, list(e2cur)))
                        while len(pend) > UDELAY:
                            emit_u(pend.pop(0))
                    if c in hooks:
                        hooks[c]()
                while pend:
                    emit_u(pend.pop(0))
                return zacc, u_lo, u_hi

            # ---- post-attention chain, split for interleaved emission ----
            def make_post(p, b, zacc, u_lo, u_hi):
                st = {}
                wf = w0_sb if b == 0 else w1_sb
                bf = b0_sb if b == 0 else b1_sb

                def a1():
                    # 1/Z and its journey to row form (no ACT ops)
                    with nc.allow_low_precision("1/Z in bf16 is plenty"):
                        rzc = smp.tile([128, 16], DT16, tag="rzc",
                                       name=f"rzc_{p}_{b}")
                        nc.vector.reciprocal(rzc[:, :], zacc[:, :])
                    zrt = pm.tile([16, 128], DT16, tag="m", name=f"zrt_{p}_{b}")
                    nc.tensor.transpose(zrt[:, :], rzc[:, :], id_sb[:, :])
                    rzt = smp.tile([16, 128], DT16, tag="rzt", name=f"rzt_{p}_{b}")
                    nc.vector.tensor_copy(rzt[:, :], zrt[:, :])
                    rzrow = smp.tile([2, 8, 128], DT16, tag="rzrow",
                                     name=f"rzrow_{p}_{b}")
                    nc.sync.dma_start(rzrow[0:1, :, :], rzt[0:8, :])
                    nc.sync.dma_start(rzrow[1:2, :, :], rzt[8:16, :])
                    st["rzrow"] = rzrow

                def a2():
                    # broadcast 1/Z across partitions, normalize U, residual,
                    # then qkv LN stats (no ACT ops)
                    rzrow2 = st["rzrow"].rearrange("s c j -> s (c j)")
                    rzb_lo = pm.tile([128, NH], F32, tag="m", name=f"rzblo_{p}_{b}")
                    rzb_hi = pm.tile([128, NH], F32, tag="m", name=f"rzbhi_{p}_{b}")
                    nc.tensor.matmul(rzb_lo[:, :], selbcb[:, :], rzrow2[:, 0:NH],
                                     start=True, stop=True)
                    nc.tensor.matmul(rzb_hi[:, :], selbcb[:, :], rzrow2[:, NH:N],
                                     start=True, stop=True)
                    # DVE reads at most one PSUM operand per instruction, so
                    # stage 1/Z in SBUF before the U multiply.
                    rzb = smp.tile([128, N], F32, tag="rzb", name=f"rzb_{p}_{b}")
                    nc.vector.tensor_copy(rzb[:, 0:NH], rzb_lo[:, :])
                    nc.vector.tensor_copy(rzb[:, NH:N], rzb_hi[:, :])
                    v = bigp.tile([128, N], DT16, tag="v", name=f"v_{p}_{b}")
                    nc.vector.tensor_tensor(v[:, 0:NH], u_lo[:, :], rzb[:, 0:NH],
                                            op=ALU.mult)
                    nc.vector.tensor_tensor(v[:, NH:N], u_hi[:, :], rzb[:, NH:N],
                                            op=ALU.mult)
                    nc.vector.tensor_tensor(v[:, :], v[:, :],
                                            pair_state[p]["xemb"][:, :], op=ALU.add)
                    st["v"] = v
                    st["mrs1"] = ln_stats(v, f"q{p}{b}")

                def bfn():
                    # qkv LN finish (2 tiny ACT ops), FFN, fc LN stats
                    qkv = bigp.tile([128, N], DT16, tag="qkv", name=f"qkv_{p}_{b}")
                    ln_finish(st["v"], st["mrs1"], qkv, f"q{p}{b}")
                    fps_lo = pm.tile([128, NH], F32, tag="m", name=f"fpslo_{p}_{b}")
                    fps_hi = pm.tile([128, NH], F32, tag="m", name=f"fpshi_{p}_{b}")
                    for fps, cols in ((fps_lo, slice(0, NH)), (fps_hi, slice(NH, N))):
                        nc.tensor.matmul(fps[0:64, :], wf[0:64, :], qkv[0:64, cols],
                                         start=True, stop=True)
                        nc.tensor.matmul(fps[64:128, :], wf[64:128, :],
                                         qkv[64:128, cols], start=True, stop=True)
                    f = bigp.tile([128, N], DT16, tag="f", name=f"f_{p}_{b}")
                    nc.vector.tensor_scalar(f[:, 0:NH], fps_lo[:, :], bf[:, :], 0.0,
                                            op0=ALU.add, op1=ALU.max)
                    nc.vector.tensor_scalar(f[:, NH:N], fps_hi[:, :], bf[:, :], 0.0,
                                            op0=ALU.add, op1=ALU.max)
                    nc.vector.tensor_tensor(f[:, :], f[:, :], qkv[:, :], op=ALU.add)
                    st["f"] = f
                    st["mrs2"] = ln_stats(f, f"f{p}{b}")

                def cfn():
                    # fc LN finish (2 tiny ACT ops), then either next-block
                    # transposes or the output head
                    fc = bigp.tile([128, N], DT16, tag="fc", name=f"fc_{p}_{b}")
                    ln_finish(st["f"], st["mrs2"], fc, f"f{p}{b}")
                    pair_state[p]["hT"] = fc
                    if b == 0:
                        pair_state[p]["hnc"] = emit_transposes(fc, f"{p}_{b + 1}")
                    else:
                        emit_head(p, fc)

                return a1, a2, bfn, cfn

            # ---- pool + head ----
            def emit_head(p, fc):
                pooled = smp.tile([128, 1], F32, tag="pool", name=f"pool_{p}")
                nc.vector.reduce_sum(pooled[:, :], fc[:, :],
                                     axis=mybir.AxisListType.X)
                sc = pm.tile([1, 2], F32, tag="m", name=f"sc_{p}")
                nc.tensor.matmul(sc[0:1, 0:1], wo_sb[0:64, :], pooled[0:64, :],
                                 start=True, stop=True, skip_group_check=True)
                nc.tensor.matmul(sc[0:1, 1:2], wo_sb[64:128, :], pooled[64:128, :],
                                 start=True, stop=True, skip_group_check=True)
                zb = smp.tile([1, 2], F32, tag="zb", name=f"zb_{p}")
                nc.vector.tensor_scalar(zb[:, :], sc[:, :], bo_sb[0:1, :], None,
                                        op0=ALU.add)
                res = smp.tile([1, 2], F32, tag="res", name=f"res_{p}")
                nc.vector.tensor_scalar(res[:, :], zb[:, :], 0.01, None,
                                        op0=ALU.mult)
                nc.vector.tensor_tensor(res[:, :], res[:, :], zb[:, :], op=ALU.max)
                for si in range(2):
                    nc.sync.dma_start(
                        out_d[2 * p + si:2 * p + si + 1, :], res[0:1, si:si + 1],
                    )

            # ---- main schedule ----
            emit_embed(0)
            load_rest_consts()
            pair_state[0]["hnc"] = emit_transposes(pair_state[0]["hT"], "0_0")
            emit_embed(1)

            phases = [(0, 0), (1, 0), (0, 1), (1, 1)]
            pending = None
            for i, (p, b) in enumerate(phases):
                hooks = {}
                if pending is not None:
                    a1, a2, bfn, cfn = pending
                    hooks[0] = a1
                    hooks[1] = a2
                    hooks[3] = bfn
                    hooks[5] = cfn
                if i == 0:
                    hooks[2] = lambda: pair_state[1].__setitem__(
                        "hnc", emit_transposes(pair_state[1]["hT"], "1_0"))
                zacc, u_lo, u_hi = emit_attention(p, b, hooks)
                pending = make_post(p, b, zacc, u_lo, u_hi)

            for fn in pending:
                fn()

    nc.compile()
    return nc


_NC_CACHE: dict = {}


def prepare_common(We, be, gamma, beta, W0, b0, W1, b1, Wout, bout, use_gb):
    ident = np.eye(128, dtype=np.float32)
    selbc = np.zeros((2, 128), dtype=np.float32)
    selbc[0, 0:64] = 1.0
    selbc[1, 64:128] = 1.0

    def stack2(v):
        v = np.asarray(v, dtype=np.float32).reshape(-1)
        return np.concatenate([v, v]).reshape(128, 1)

    common = {
        "We": np.ascontiguousarray(np.asarray(We, dtype=np.float32)).reshape(
            2, 128, H).astype(np.float16),
        "W0s": np.concatenate([W0, W0]).astype(np.float16),
        "W1s": np.concatenate([W1, W1]).astype(np.float16),
        "Wouts": np.concatenate([Wout, Wout]).astype(np.float32),
        "be2": stack2(be), "b02": stack2(b0), "b12": stack2(b1),
        "bout": np.asarray(bout, dtype=np.float32).reshape(1, 1),
        "ident": ident.astype(np.float16),
        "selbcb": selbc.astype(np.float16),
        "allsel": (np.kron(np.eye(2, dtype=np.float32),
                           np.ones((64, 64), dtype=np.float32)) / 64.0),
    }
    if use_gb:
        gT = np.ascontiguousarray(np.asarray(gamma, dtype=np.float32).T)
        bT = np.ascontiguousarray(np.asarray(beta, dtype=np.float32).T)
        common["gT2"] = np.concatenate([gT, gT]).astype(np.float32)
        common["bT2"] = np.concatenate([bT, bT]).astype(np.float32)
    return common


def prepare_core_map(common, x, mask, k, use_mask):
    xs = x[S * k:S * k + S]                       # [S, N, F_IN]
    xTs = np.ascontiguousarray(xs.transpose(0, 2, 1)).reshape(S, 2, 128, N)
    m = dict(common)
    m["xT"] = xTs.astype(np.float16)
    if use_mask:
        m["maskN"] = np.ascontiguousarray(mask[S * k:S * k + S])
    return m


def kernel(x, mask, We, be, gamma, beta, W0, b0, W1, b1, Wout, bout):
    x = np.ascontiguousarray(np.asarray(x, dtype=np.float32))
    mask = np.asarray(mask, dtype=np.float32)
    use_mask = bool(np.any(mask))
    use_gb = bool(np.any(np.asarray(gamma) != 1.0) or np.any(np.asarray(beta)))

    key = (use_mask, use_gb)
    if key not in _NC_CACHE:
        _NC_CACHE[key] = build_nc(use_mask, use_gb)
    nc = _NC_CACHE[key]

    common = prepare_common(We, be, gamma, beta, W0, b0, W1, b1, Wout, bout,
                            use_gb)
    in_maps = [prepare_core_map(common, x, mask, k, use_mask)
               for k in range(NCORES)]

    res = run_bass_kernel_spmd(nc, in_maps, list(range(NCORES)))
    global LAST_RESULT
    LAST_RESULT = res
    out = np.concatenate([res.results[k]["out"] for k in range(NCORES)], axis=0)
    return out.astype(np.float32)


LAST_RESULT = None
